# revision 27
# baseline (speedup 1.0000x reference)
"""Tensor-parallel decoder layer (RMSNorm + RoPE causal attention + SwiGLU MLP)
for 8 Trainium2 NeuronCores.

Sharding: q/k/v and gate/up column-sharded (2 heads, 1024 ffn dims per core),
wo/down row-sharded with fp16 AllReduces. Attention scores are computed
transposed (S^T) on the PE so probabilities feed the PV matmul directly with
no transposes; softmax row-sums come from a ones-vector matmul and the
normalization is applied to the 128x512 output tile.

RMSNorm-1 is folded into the host-prepared input (xn = x/rms(x)); the
attention residual is folded into AllReduce-1 (each core adds x/8 to its Wo
partial, so a1 = x + attn_out), and the final output is y = a1 + a2.

kernel(**inputs) takes full unsharded inputs, returns the full output.
"""

import math
import numpy as np
from contextlib import ExitStack

import concourse.bass as bass
import concourse.mybir as mybir
import concourse.tile as tile
from concourse import bacc, bass_utils

f32 = mybir.dt.float32
f16 = mybir.dt.float16

NCORES = 8
P = 128
TCH = 512
BASE = 10000.0
EPS = 1e-6
EXP_BIAS = -4.0

B, T, D, H, FF = 2, 2048, 2048, 16, 8192
HD = D // H              # 128
N = B * T                # 4096
NH = H // NCORES         # heads per core = 2
DH = NH * P              # 256
FH = FF // NCORES        # 1024
KD = D // P              # 16
KF = FH // P             # 8
CC = T // TCH            # 4 chunks per batch
NTC = N // TCH           # 8 chunks total
NAR = 4                  # all-reduce chunks
GPA = NTC // NAR         # 2 chunks per AR
QT = T // P              # 16 key tiles per batch

AF = mybir.ActivationFunctionType
ALU = mybir.AluOpType


def build_decoder():
    rgroups = [list(range(NCORES))]
    nc = bacc.Bacc("TRN2", target_bir_lowering=False, debug=False,
                   num_devices=NCORES)

    # ---- I/O (host-tiled layouts) ----
    xn16 = nc.dram_tensor("xn16", [KD, P, N], f16, kind="ExternalInput")
    xo8 = nc.dram_tensor("xo8", [KD, P, N], f16, kind="ExternalInput")
    cq = nc.dram_tensor("cq", [P, N], f16, kind="ExternalInput")
    sq = nc.dram_tensor("sq", [P, N], f16, kind="ExternalInput")
    ck = nc.dram_tensor("ck", [P, N], f16, kind="ExternalInput")
    sk = nc.dram_tensor("sk", [P, N], f16, kind="ExternalInput")
    maskd = nc.dram_tensor("maskd", [P, P], f32, kind="ExternalInput")
    wqkv = nc.dram_tensor("wqkv", [P, KD, 3 * DH], f16, kind="ExternalInput")
    wo = nc.dram_tensor("wo", [P, NH, D], f16, kind="ExternalInput")
    wg = nc.dram_tensor("wg", [P, KD, FH], f16, kind="ExternalInput")
    wu = nc.dram_tensor("wu", [P, KD, FH], f16, kind="ExternalInput")
    wd = nc.dram_tensor("wd", [P, KF, D], f16, kind="ExternalInput")
    yT = nc.dram_tensor("yT", [NTC, KD, P, TCH], f16, kind="ExternalOutput")

    p1 = [nc.dram_tensor(f"p1_{a}", [KD, P, TCH], f16) for a in range(NTC)]
    a1 = [nc.dram_tensor(f"a1_{a}", [KD, P, TCH], f16, addr_space="Shared")
          for a in range(NTC)]
    p2 = [nc.dram_tensor(f"p2_{a}", [KD, P, TCH], f16) for a in range(NTC)]
    a2 = [nc.dram_tensor(f"a2_{a}", [KD, P, TCH], f16, addr_space="Shared")
          for a in range(NTC)]
    xn2d = nc.dram_tensor("xn2d", [NTC, KD, P, TCH], f16)

    with tile.TileContext(nc, pool_alloc_mode="queue") as tc, ExitStack() as ctx:
        constp = ctx.enter_context(tc.tile_pool(name="constp", bufs=1))
        ones_k = constp.tile([P, 1], f16)
        nc.vector.memset(ones_k, 1.0)
        ones_1 = constp.tile([1, P], f16)
        nc.vector.memset(ones_1, 1.0)
        ebias = constp.tile([P, 1], f32)
        nc.vector.memset(ebias, EXP_BIAS)
        eps1 = constp.tile([1, 1], f32)
        nc.vector.memset(eps1, EPS)
        mask_sb = constp.tile([P, P], f32)
        nc.sync.dma_start(mask_sb, maskd[:, :])

        # LEFT stack, bottom-up: constp, cpool, persist1, persist0 — pops
        # (persist0 mid-B, persist1 end-B, cpool at end) stay LIFO.
        cpool = tc.alloc_tile_pool(name="cpool", bufs=1)
        persist = [None, None]
        persist[1] = tc.alloc_tile_pool(name="persist1", bufs=1)
        persist[0] = tc.alloc_tile_pool(name="persist0", bufs=1)
        qk_f = [[persist[b_].tile([P, T], f16, name=f"qkf{b_}_{m}",
                                  tag=f"qkf{m}")
                 for m in range(2 * NH)] for b_ in range(2)]
        v_sb = [[persist[b_].tile([P, T], f16, name=f"vsb{b_}_{h}",
                                  tag=f"vsb{h}")
                 for h in range(NH)] for b_ in range(2)]

        # A-phase pools on the RIGHT stack (released after A)
        awp = tc.alloc_tile_pool(name="awp", bufs=1, side="right")
        wqkv_sb = awp.tile([P, KD, 3 * DH], f16, name="wqkv_sb", tag="wqkv")
        nc.sync.dma_start(wqkv_sb, wqkv[:, :, :])

        axp = tc.alloc_tile_pool(name="axp", bufs=1, side="right")
        psA = tc.alloc_tile_pool(name="psA", bufs=1, space="PSUM")

        # ================= A: QKV projections + RoPE =================
        for half in range(2):
            for cc in range(CC):
                gc = half * CC + cc
                gsl = slice(gc * TCH, (gc + 1) * TCH)
                csl = slice(cc * TCH, (cc + 1) * TCH)
                xc = axp.tile([P, KD, TCH], f16, name="xac", tag="xac",
                              bufs=3)
                nc.sync.dma_start(
                    xc, xn16.ap()[:, :, gsl].rearrange("k p t -> p k t"))
                tabt = {}
                for nm, dram in (("cq", cq), ("sq", sq), ("ck", ck),
                                 ("sk", sk)):
                    tt = axp.tile([P, TCH], f16, name=nm, tag=f"tab{nm}",
                                  bufs=2)
                    nc.sync.dma_start(tt, dram[:, gsl])
                    tabt[nm] = tt
                for m in range(3 * NH):
                    qk = psA.tile([P, TCH], f32, name="qk", tag="qk", bufs=4)
                    for i in range(KD):
                        nc.tensor.matmul(qk, wqkv_sb[:, i, m * P:(m + 1) * P],
                                         xc[:, i, :],
                                         start=(i == 0), stop=(i == KD - 1))
                    if m < 2 * NH:
                        # q or k head: evict + rope
                        isq = m < NH
                        ct = tabt["cq"] if isq else tabt["ck"]
                        st = tabt["sq"] if isq else tabt["sk"]
                        qh = axp.tile([P, TCH], f16, name="qh", tag="qh",
                                      bufs=3)
                        nc.scalar.copy(qh, qk)
                        t1 = axp.tile([P, TCH], f16, name="t1", tag="t1",
                                      bufs=2)
                        nc.vector.tensor_mul(t1, qh, ct)
                        qp_t = axp.tile([P, TCH], f16, name="qp", tag="qp",
                                        bufs=3)
                        nc.sync.dma_start(qp_t[0:P // 2, :], qh[P // 2:P, :])
                        nc.sync.dma_start(qp_t[P // 2:P, :], qh[0:P // 2, :])
                        t2 = axp.tile([P, TCH], f16, name="t2", tag="t2",
                                      bufs=2)
                        nc.vector.tensor_mul(t2, qp_t, st)
                        nc.vector.tensor_add(qk_f[half][m][:, csl], t1, t2)
                    else:
                        h = m - 2 * NH
                        vtr = axp.tile([P, TCH], f16, name="vtr", tag="vtr",
                                       bufs=2)
                        nc.scalar.copy(vtr, qk)
                        for jj in range(TCH // P):
                            kt = cc * (TCH // P) + jj
                            nc.sync.dma_start(
                                v_sb[half][h][:, kt * P:(kt + 1) * P],
                                vtr[:, jj * P:(jj + 1) * P], transpose=True)
        psA.release()
        axp.release()
        awp.release()

        # B/C pools; mwp_w (MLP weights) under attp so attp can pop first
        psC = tc.alloc_tile_pool(name="psC", bufs=1, space="PSUM")
        psB = tc.alloc_tile_pool(name="psB", bufs=1, space="PSUM")
        mwp_w = tc.alloc_tile_pool(name="mwp_w", bufs=1, side="right")
        attp = tc.alloc_tile_pool(name="attp", bufs=1, side="right")
        wo_sb = cpool.tile([P, NH, D], f16, name="wo_sb", tag="wo_sb")
        nc.sync.dma_start(wo_sb, wo[:, :, :])

        o_q = {}

        def stage_B_attn(b_, qc):
            """Attention (S^T) for one 512 chunk — SBUF-only, no DMA."""
            gc = b_ * CC + qc
            nkt = 4 * (qc + 1)
            # scores^T + exp, both heads
            eT = [[], []]
            for h in range(NH):
                for kt in range(nkt):
                    sc = psB.tile([P, TCH], f32, name="sc", tag="sc", bufs=2)
                    nc.tensor.matmul(
                        sc, qk_f[b_][NH + h][:, kt * P:(kt + 1) * P],
                        qk_f[b_][h][:, qc * TCH:(qc + 1) * TCH],
                        start=True, stop=True)
                    j = kt - qc * 4
                    e = attp.tile([P, TCH], f16, name="e", tag="e", bufs=16)
                    if j >= 0:
                        nc.vector.tensor_add(sc[:, j * P:(j + 1) * P],
                                             sc[:, j * P:(j + 1) * P],
                                             mask_sb)
                        if j > 0:
                            nc.vector.memset(e[:, 0:j * P], 0.0)
                        nc.scalar.activation(e[:, j * P:], sc[:, j * P:],
                                             AF.Exp, bias=ebias[:, :],
                                             scale=1.0)
                    else:
                        nc.scalar.activation(e, sc, AF.Exp, bias=ebias[:, :],
                                             scale=1.0)
                    eT[h].append(e)
            # rowsum + PV + normalize per head
            o_chunk = []
            for h in range(NH):
                rs = psB.tile([1, TCH], f32, name="rs", tag="rs", bufs=1)
                for kt in range(nkt):
                    nc.tensor.matmul(rs, ones_k, eT[h][kt],
                                     start=(kt == 0), stop=(kt == nkt - 1))
                op = psB.tile([P, TCH], f32, name="op", tag="op", bufs=1)
                for kt in range(nkt):
                    nc.tensor.matmul(op, v_sb[b_][h][:, kt * P:(kt + 1) * P],
                                     eT[h][kt],
                                     start=(kt == 0), stop=(kt == nkt - 1))
                rcp = cpool.tile([1, TCH], f32, name="rcp", tag="rcp", bufs=2)
                nc.vector.reciprocal(rcp, rs)
                rcp16 = cpool.tile([1, TCH], f16, name="rcp16", tag="rcp16",
                                   bufs=2)
                nc.vector.tensor_scalar_mul(rcp16, rcp, 1.0)
                rbp = psB.tile([P, TCH], f32, name="rbp", tag="sc", bufs=2)
                nc.tensor.matmul(rbp, ones_1, rcp16, start=True, stop=True)
                rbv = attp.tile([P, TCH], f16, name="rbv", tag="rbv", bufs=2)
                nc.vector.tensor_scalar_mul(rbv, rbp, 1.0)
                ob = attp.tile([P, TCH], f16, name="ob", tag="ob", bufs=6)
                nc.vector.tensor_tensor(ob, op, rbv, ALU.mult)
                o_chunk.append(ob)
            o_q[gc] = o_chunk

        def stage_B_wo(b_, qc):
            """Wo partials + x/8 fold, quarter-staged writes to p1."""
            gc = b_ * CC + qc
            o_chunk = o_q.pop(gc)
            for mq in range(4):
                xo8c = attp.tile([P, 4, TCH], f16, name="xo8c", tag="xo8c",
                                 bufs=2)
                nc.sync.dma_start(
                    xo8c, xo8.ap()[mq * 4:(mq + 1) * 4, :,
                                   gc * TCH:(gc + 1) * TCH]
                    .rearrange("k p t -> p k t"))
                p1s = attp.tile([P, 4, TCH], f16, name="p1s", tag="p1s",
                                bufs=3)
                for mi in range(4):
                    mout = mq * 4 + mi
                    wop = psB.tile([P, TCH], f32, name="wop", tag="wop",
                                   bufs=2)
                    for h in range(NH):
                        nc.tensor.matmul(
                            wop, wo_sb[:, h, mout * P:(mout + 1) * P],
                            o_chunk[h], start=(h == 0), stop=(h == NH - 1))
                    nc.vector.tensor_tensor(p1s[:, mi, :], wop,
                                            xo8c[:, mi, :], ALU.add)
                nc.sync.dma_start(
                    p1[gc].ap()[mq * 4:(mq + 1) * 4, :, :]
                    .rearrange("k p t -> p k t"), p1s)

        def stage_AR1(gc):
            nc.gpsimd.collective_compute(
                "AllReduce", ALU.add, replica_groups=rgroups,
                ins=[p1[gc].ap()], outs=[a1[gc].ap()])

        def stage_C(gc):
            """x1 = a1 chunk -> rms2 -> write normalized xn2 tiles."""
            a1c = cpool.tile([P, KD, TCH], f16, name="a1c", tag="a1c", bufs=1)
            nc.sync.dma_start(
                a1c, a1[gc].ap()[:, :, :].rearrange("k p t -> p k t"))
            ssq = psC.tile([1, TCH], f32, name="ssq", tag="ssq", bufs=1)
            for i in range(KD):
                x2t = cpool.tile([P, TCH], f16, name="x2t", tag="x2t", bufs=2)
                nc.vector.tensor_mul(x2t, a1c[:, i, :], a1c[:, i, :])
                nc.tensor.matmul(ssq, ones_k, x2t,
                                 start=(i == 0), stop=(i == KD - 1))
            srt = cpool.tile([1, TCH], f32, name="srt", tag="srt", bufs=1)
            nc.scalar.activation(srt, ssq, AF.Sqrt, bias=eps1[:, :],
                                 scale=1.0 / D)
            rr2 = cpool.tile([1, TCH], f32, name="rr2", tag="rr2", bufs=1)
            nc.vector.reciprocal(rr2, srt)
            rr216 = cpool.tile([1, TCH], f16, name="rr216", tag="rr216",
                               bufs=2)
            nc.vector.tensor_scalar_mul(rr216, rr2, 1.0)
            rbp2 = psC.tile([P, TCH], f32, name="rbp2", tag="rbp2", bufs=1)
            nc.tensor.matmul(rbp2, ones_1, rr216, start=True, stop=True)
            rsb = cpool.tile([P, TCH], f16, name="rsb", tag="rsb", bufs=2)
            nc.vector.tensor_scalar_mul(rsb, rbp2, 1.0)
            for kq in range(4):
                xnc = cpool.tile([P, 4, TCH], f16, name="xnc", tag="xnc",
                                 bufs=2)
                for ki in range(4):
                    i = kq * 4 + ki
                    nc.gpsimd.tensor_tensor(xnc[:, ki, :], a1c[:, i, :],
                                            rsb, ALU.mult)
                nc.sync.dma_start(
                    xn2d.ap()[gc, kq * 4:(kq + 1) * 4, :, :]
                    .rearrange("k p t -> p k t"), xnc)

        xn2q = {}

        def load_D(cc):
            xc = dwp2.tile([P, KD, TCH], f16, name="xn2c", tag="xn2c",
                           bufs=2)
            nc.sync.dma_start(
                xc, xn2d.ap()[cc, :, :, :].rearrange("k p t -> p k t"))
            xn2q[cc] = xc

        def stage_D(cc):
            """MLP for one 512-token chunk; p2 partial (mlp only)."""
            xc = xn2q.pop(cc)
            acs = []
            for fm in range(KF):
                gp = psD.tile([P, TCH], f32, name="gp", tag="gp", bufs=2)
                for i in range(KD):
                    nc.tensor.matmul(gp, wg_sb[:, i, fm * P:(fm + 1) * P],
                                     xc[:, i, :],
                                     start=(i == 0), stop=(i == KD - 1))
                up = psD.tile([P, TCH], f32, name="up", tag="up", bufs=2)
                for i in range(KD):
                    nc.tensor.matmul(up, wu_sb[:, i, fm * P:(fm + 1) * P],
                                     xc[:, i, :],
                                     start=(i == 0), stop=(i == KD - 1))
                gss = dwp1.tile([P, TCH], f16, name="gss", tag="gss", bufs=2)
                nc.scalar.activation(gss, gp, AF.Silu)
                ac = dwp2.tile([P, TCH], f16, name="ac", tag="ac",
                               bufs=KF + 1)
                nc.vector.tensor_tensor(ac, gss, up, ALU.mult)
                acs.append(ac)
            for mq in range(4):
                p2s = dwp1.tile([P, 4, TCH], f16, name="p2s", tag="p2s",
                               bufs=2)
                for mi in range(4):
                    mout = mq * 4 + mi
                    dp = psD.tile([P, TCH], f32, name="dp", tag="dp", bufs=2)
                    for fi in range(KF):
                        nc.tensor.matmul(
                            dp, wd_sb[:, fi, mout * P:(mout + 1) * P],
                            acs[fi], start=(fi == 0), stop=(fi == KF - 1))
                    nc.vector.tensor_scalar_mul(p2s[:, mi, :], dp, 1.0)
                nc.sync.dma_start(
                    p2[cc].ap()[mq * 4:(mq + 1) * 4, :, :]
                    .rearrange("k p t -> p k t"), p2s)

        def stage_AR2(cc):
            nc.gpsimd.collective_compute(
                "AllReduce", ALU.add, replica_groups=rgroups,
                ins=[p2[cc].ap()], outs=[a2[cc].ap()])

        def stage_Y(cc):
            """y = a1 + a2 for one 512 chunk -> yT."""
            for hh in range(8):
                ksl = slice(hh * 2, (hh + 1) * 2)
                ay1 = ypool.tile([P, 2, TCH], f16, name="ay1", tag="ay1",
                                 bufs=1)
                nc.sync.dma_start(
                    ay1, a1[cc].ap()[ksl, :, :].rearrange("k p t -> p k t"))
                ay2 = ypool.tile([P, 2, TCH], f16, name="ay2", tag="ay2",
                                 bufs=1)
                nc.sync.dma_start(
                    ay2, a2[cc].ap()[ksl, :, :].rearrange("k p t -> p k t"))
                yc = ypool.tile([P, 2, TCH], f16, name="yc", tag="yc",
                                bufs=1)
                nc.gpsimd.tensor_tensor(yc, ay1, ay2, ALU.add)
                nc.sync.dma_start(
                    yT.ap()[cc, ksl, :, :].rearrange("k p t -> p k t"), yc)

        def stage_Y_big(cc):
            """Final-chunk y using the (now idle) wide xn2c buffers."""
            ay1 = dwp2.tile([P, KD, TCH], f16, name="yb1", tag="xn2c",
                            bufs=2)
            nc.sync.dma_start(
                ay1, a1[cc].ap()[:, :, :].rearrange("k p t -> p k t"))
            ay2 = dwp2.tile([P, KD, TCH], f16, name="yb2", tag="xn2c",
                            bufs=2)
            nc.sync.dma_start(
                ay2, a2[cc].ap()[:, :, :].rearrange("k p t -> p k t"))
            nc.gpsimd.tensor_tensor(ay1, ay1, ay2, ALU.add)
            nc.sync.dma_start(
                yT.ap()[cc, :, :, :].rearrange("k p t -> p k t"), ay1)

        # ================= pipelined issue order =================
        # Each per-chunk AllReduce ring (~40us of saturated DMA engines)
        # is covered by the next chunk's DMA-free attention compute.
        stage_B_attn(0, 0)
        stage_B_wo(0, 0)
        stage_AR1(0)
        stage_B_attn(0, 1)
        stage_B_wo(0, 1)
        stage_AR1(1)
        stage_B_attn(0, 2)
        # kick gate/up weight loads
        wg_sb = mwp_w.tile([P, KD, FH], f16, name="wg_sb", tag="wg_sb")
        nc.sync.dma_start(wg_sb, wg[:, :, :])
        wu_sb = mwp_w.tile([P, KD, FH], f16, name="wu_sb", tag="wu_sb")
        nc.sync.dma_start(wu_sb, wu[:, :, :])
        stage_B_wo(0, 2)
        stage_AR1(2)
        stage_B_attn(0, 3)
        stage_B_wo(0, 3)
        stage_AR1(3)
        persist[0].release()

        stage_B_attn(1, 0)
        stage_B_wo(1, 0)
        stage_AR1(4)
        stage_C(0)
        stage_B_attn(1, 1)
        stage_B_wo(1, 1)
        stage_AR1(5)
        stage_C(1)
        stage_B_attn(1, 2)
        stage_B_wo(1, 2)
        stage_AR1(6)
        stage_C(2)
        stage_B_attn(1, 3)
        stage_B_wo(1, 3)
        stage_AR1(7)
        stage_C(3)
        persist[1].release()
        psB.release()
        attp.release()
        psD = tc.alloc_tile_pool(name="psD", bufs=1, space="PSUM")
        dwp1 = tc.alloc_tile_pool(name="dwp1", bufs=1, side="right")
        dwp2 = tc.alloc_tile_pool(name="dwp2", bufs=1, side="right")
        ypool = tc.alloc_tile_pool(name="ypool", bufs=1, side="right")
        wd_sb = dwp1.tile([P, KF, D], f16, name="wd_sb", tag="wd_sb")
        nc.sync.dma_start(wd_sb, wd[:, :, :])

        load_D(0)
        load_D(1)
        stage_D(0)
        stage_AR2(0)
        stage_C(4)
        load_D(2)
        stage_D(1)
        stage_AR2(1)
        stage_C(5)
        load_D(3)
        stage_D(2)
        stage_AR2(2)
        stage_C(6)
        load_D(4)
        stage_D(3)
        stage_AR2(3)
        stage_C(7)
        load_D(5)
        stage_Y(0)
        stage_D(4)
        stage_AR2(4)
        load_D(6)
        stage_Y(1)
        stage_D(5)
        stage_AR2(5)
        load_D(7)
        stage_Y(2)
        stage_D(6)
        stage_AR2(6)
        stage_Y(3)
        stage_Y(4)
        stage_D(7)
        stage_AR2(7)
        stage_Y(5)
        stage_Y(6)
        stage_Y_big(7)

        psD.release()
        psC.release()
        ypool.release()
        dwp2.release()
        dwp1.release()
        mwp_w.release()
        cpool.release()

    nc.compile()
    return nc


# ---------------- host side ----------------

_BUILT = {}


def _get_program():
    if "full" not in _BUILT:
        _BUILT["full"] = build_decoder()
    return _BUILT["full"]


def _tile_k(a2d):
    """[D, N] -> [KD, P, N] contiguous f16."""
    return np.ascontiguousarray(
        a2d.reshape(KD, P, a2d.shape[1])).astype(np.float16)


def _host_prep(x, position_ids, Wq, Wk, Wv, Wo, Wg, Wu, Wd, g1, g2):
    xf = np.asarray(x, np.float32).reshape(N, D)
    rs1 = 1.0 / np.sqrt(np.mean(xf * xf, axis=1, keepdims=True) + EPS)
    xn = (xf * rs1).T                       # [D, N] normalized (g1 in weights)
    xo8f = (xf.T / 8.0)                     # [D, N]
    xn16 = _tile_k(xn)
    xo8t = _tile_k(xo8f)

    pos = np.asarray(position_ids).reshape(-1).astype(np.float32)
    inv_freq = (1.0 / (BASE ** (np.arange(0, HD, 2, dtype=np.float32) / HD)))
    ang = pos[:, None] * inv_freq[None, :]
    cos_f = np.concatenate([np.cos(ang), np.cos(ang)], axis=1)   # [N, HD]
    sin_f = np.concatenate([np.sin(ang), np.sin(ang)], axis=1)
    sign = np.concatenate([-np.ones(HD // 2, np.float32),
                           np.ones(HD // 2, np.float32)])
    s = 1.0 / math.sqrt(HD)
    cqt = np.ascontiguousarray(cos_f.T * s).astype(np.float16)
    sqt = np.ascontiguousarray((sin_f * sign).T * s).astype(np.float16)
    ckt = np.ascontiguousarray(cos_f.T).astype(np.float16)
    skt = np.ascontiguousarray((sin_f * sign).T).astype(np.float16)

    ii, jj = np.indices((P, P))
    maskv = np.where(ii > jj, np.float32(-10000.0), np.float32(0.0))

    g1f = np.asarray(g1, np.float32)[:, None]
    g2f = np.asarray(g2, np.float32)[:, None]
    wqs = (g1f * np.asarray(Wq, np.float32)).astype(np.float16)
    wks = (g1f * np.asarray(Wk, np.float32)).astype(np.float16)
    wvs = (g1f * np.asarray(Wv, np.float32)).astype(np.float16)
    wgs = (g2f * np.asarray(Wg, np.float32)).astype(np.float16)
    wus = (g2f * np.asarray(Wu, np.float32)).astype(np.float16)
    wds = np.asarray(Wd, np.float32).astype(np.float16)
    wos = np.asarray(Wo, np.float32).astype(np.float16)

    def tile_w(w):   # [D_in, M] -> [P, D_in//P, M]
        din, m = w.shape
        return np.ascontiguousarray(
            w.reshape(din // P, P, m).transpose(1, 0, 2))

    in_maps = []
    for i in range(NCORES):
        qs, fs = slice(i * DH, (i + 1) * DH), slice(i * FH, (i + 1) * FH)
        wq_c = np.concatenate([wqs[:, qs], wks[:, qs], wvs[:, qs]], axis=1)
        wo_c = wos[qs, :]                    # [DH, D]
        in_maps.append({
            "xn16": xn16, "xo8": xo8t,
            "cq": cqt, "sq": sqt, "ck": ckt, "sk": skt,
            "maskd": maskv,
            "wqkv": tile_w(wq_c),
            "wo": np.ascontiguousarray(
                wo_c.reshape(NH, P, D).transpose(1, 0, 2)),
            "wg": tile_w(wgs[:, fs]),
            "wu": tile_w(wus[:, fs]),
            "wd": tile_w(wds[fs, :]),
        })
    return in_maps


FULL_CFG = dict(B=B, T=T, D=D, H=H, FF=FF)


def run(cfg, inputs, **run_kwargs):
    nc = _get_program()
    in_maps = _host_prep(**inputs)
    res = bass_utils.run_bass_kernel_spmd(
        nc, in_maps, core_ids=list(range(NCORES)), **run_kwargs)
    yt = res.results[0]["yT"]               # [NTC, KD, P, TCH] f16
    y = np.transpose(yt.astype(np.float32), (0, 3, 1, 2)).reshape(N, D)
    return y.reshape(B, T, D), res


def kernel(**inputs):
    y, _ = run(FULL_CFG, inputs)
    return y


# revision 29
# speedup vs baseline: 1.0470x; 1.0470x over previous
"""Tensor-parallel decoder layer (RMSNorm + RoPE causal attention + SwiGLU MLP)
for 8 Trainium2 NeuronCores.

Sharding: q/k/v and gate/up column-sharded (2 heads, 1024 ffn dims per core),
wo/down row-sharded with fp16 AllReduces. Attention scores are computed
transposed (S^T) on the PE so probabilities feed the PV matmul directly with
no transposes; softmax row-sums come from a ones-vector matmul and the
normalization is applied to the 128x512 output tile.

RMSNorm-1 is folded into the host-prepared input (xn = x/rms(x)); the
attention residual is folded into AllReduce-1 (each core adds x/8 to its Wo
partial, so a1 = x + attn_out), and the final output is y = a1 + a2.

kernel(**inputs) takes full unsharded inputs, returns the full output.
"""

import math
import numpy as np
from contextlib import ExitStack

import concourse.bass as bass
import concourse.mybir as mybir
import concourse.tile as tile
from concourse import bacc, bass_utils

f32 = mybir.dt.float32
f16 = mybir.dt.float16

NCORES = 8
P = 128
TCH = 512
BASE = 10000.0
EPS = 1e-6
EXP_BIAS = -4.0

B, T, D, H, FF = 2, 2048, 2048, 16, 8192
HD = D // H              # 128
N = B * T                # 4096
NH = H // NCORES         # heads per core = 2
DH = NH * P              # 256
FH = FF // NCORES        # 1024
KD = D // P              # 16
KF = FH // P             # 8
CC = T // TCH            # 4 chunks per batch
NTC = N // TCH           # 8 chunks total
NAR = 4                  # all-reduce chunks
GPA = NTC // NAR         # 2 chunks per AR
QT = T // P              # 16 key tiles per batch

AF = mybir.ActivationFunctionType
ALU = mybir.AluOpType


def build_decoder():
    rgroups = [list(range(NCORES))]
    nc = bacc.Bacc("TRN2", target_bir_lowering=False, debug=False,
                   num_devices=NCORES)

    # ---- I/O (host-tiled layouts) ----
    xn16 = nc.dram_tensor("xn16", [KD, P, N], f16, kind="ExternalInput")
    xo8 = nc.dram_tensor("xo8", [KD, P, N], f16, kind="ExternalInput")
    cq = nc.dram_tensor("cq", [P, N], f16, kind="ExternalInput")
    sq = nc.dram_tensor("sq", [P, N], f16, kind="ExternalInput")
    ck = nc.dram_tensor("ck", [P, N], f16, kind="ExternalInput")
    sk = nc.dram_tensor("sk", [P, N], f16, kind="ExternalInput")
    maskd = nc.dram_tensor("maskd", [P, P], f32, kind="ExternalInput")
    wqkv = nc.dram_tensor("wqkv", [P, KD, 3 * DH], f16, kind="ExternalInput")
    wo = nc.dram_tensor("wo", [P, NH, D], f16, kind="ExternalInput")
    wg = nc.dram_tensor("wg", [P, KD, FH], f16, kind="ExternalInput")
    wu = nc.dram_tensor("wu", [P, KD, FH], f16, kind="ExternalInput")
    wd = nc.dram_tensor("wd", [P, KF, D], f16, kind="ExternalInput")
    yT = nc.dram_tensor("yT", [NTC, KD, P, TCH], f16, kind="ExternalOutput")

    p1 = [nc.dram_tensor(f"p1_{a}", [KD, GPA, P, TCH], f16) for a in range(NAR)]
    a1 = [nc.dram_tensor(f"a1_{a}", [KD, GPA, P, TCH], f16, addr_space="Shared")
          for a in range(NAR)]
    p2 = [nc.dram_tensor(f"p2_{a}", [KD, P, TCH], f16) for a in range(NTC)]
    a2 = [nc.dram_tensor(f"a2_{a}", [KD, P, TCH], f16, addr_space="Shared")
          for a in range(NTC)]
    xn2d = nc.dram_tensor("xn2d", [NTC, KD, P, TCH], f16)

    with tile.TileContext(nc, pool_alloc_mode="queue") as tc, ExitStack() as ctx:
        constp = ctx.enter_context(tc.tile_pool(name="constp", bufs=1))
        ones_k = constp.tile([P, 1], f16)
        nc.vector.memset(ones_k, 1.0)
        ones_1 = constp.tile([1, P], f16)
        nc.vector.memset(ones_1, 1.0)
        ebias = constp.tile([P, 1], f32)
        nc.vector.memset(ebias, EXP_BIAS)
        eps1 = constp.tile([1, 1], f32)
        nc.vector.memset(eps1, EPS)
        mask_sb = constp.tile([P, P], f32)
        nc.sync.dma_start(mask_sb, maskd[:, :])

        # LEFT stack, bottom-up: constp, cpool, persist1, persist0 — pops
        # (persist0 mid-B, persist1 end-B, cpool at end) stay LIFO.
        cpool = tc.alloc_tile_pool(name="cpool", bufs=1)
        persist = [None, None]
        persist[1] = tc.alloc_tile_pool(name="persist1", bufs=1)
        persist[0] = tc.alloc_tile_pool(name="persist0", bufs=1)
        qk_f = [[persist[b_].tile([P, T], f16, name=f"qkf{b_}_{m}",
                                  tag=f"qkf{m}")
                 for m in range(2 * NH)] for b_ in range(2)]
        v_sb = [[persist[b_].tile([P, T], f16, name=f"vsb{b_}_{h}",
                                  tag=f"vsb{h}")
                 for h in range(NH)] for b_ in range(2)]

        # A-phase pools on the RIGHT stack (released after A)
        awp = tc.alloc_tile_pool(name="awp", bufs=1, side="right")
        wqkv_sb = awp.tile([P, KD, 3 * DH], f16, name="wqkv_sb", tag="wqkv")
        nc.sync.dma_start(wqkv_sb, wqkv[:, :, :])

        axp = tc.alloc_tile_pool(name="axp", bufs=1, side="right")
        psA = tc.alloc_tile_pool(name="psA", bufs=1, space="PSUM")

        # ================= A: QKV projections + RoPE =================
        a_q = {}

        def load_A(gc):
            gsl = slice(gc * TCH, (gc + 1) * TCH)
            xc = axp.tile([P, KD, TCH], f16, name="xac", tag="xac", bufs=3)
            nc.sync.dma_start(
                xc, xn16.ap()[:, :, gsl].rearrange("k p t -> p k t"))
            tabt = {}
            for nm, dram in (("cq", cq), ("sq", sq), ("ck", ck), ("sk", sk)):
                tt = axp.tile([P, TCH], f16, name=nm, tag=f"tab{nm}", bufs=3)
                nc.sync.dma_start(tt, dram[:, gsl])
                tabt[nm] = tt
            a_q[gc] = (xc, tabt)

        load_A(0)
        load_A(1)
        for half in range(2):
            for cc in range(CC):
                gc = half * CC + cc
                csl = slice(cc * TCH, (cc + 1) * TCH)
                if gc + 2 < NTC:
                    load_A(gc + 2)
                xc, tabt = a_q.pop(gc)
                for m in range(3 * NH):
                    qk = psA.tile([P, TCH], f32, name="qk", tag="qk", bufs=4)
                    for i in range(KD):
                        nc.tensor.matmul(qk, wqkv_sb[:, i, m * P:(m + 1) * P],
                                         xc[:, i, :],
                                         start=(i == 0), stop=(i == KD - 1))
                    if m < 2 * NH:
                        # q or k head: evict + rope
                        isq = m < NH
                        ct = tabt["cq"] if isq else tabt["ck"]
                        st = tabt["sq"] if isq else tabt["sk"]
                        qh = axp.tile([P, TCH], f16, name="qh", tag="qh",
                                      bufs=3)
                        nc.scalar.copy(qh, qk)
                        t1 = axp.tile([P, TCH], f16, name="t1", tag="t1",
                                      bufs=2)
                        nc.vector.tensor_mul(t1, qh, ct)
                        qp_t = axp.tile([P, TCH], f16, name="qp", tag="qp",
                                        bufs=3)
                        nc.sync.dma_start(qp_t[0:P // 2, :], qh[P // 2:P, :])
                        nc.sync.dma_start(qp_t[P // 2:P, :], qh[0:P // 2, :])
                        t2 = axp.tile([P, TCH], f16, name="t2", tag="t2",
                                      bufs=2)
                        nc.vector.tensor_mul(t2, qp_t, st)
                        nc.vector.tensor_add(qk_f[half][m][:, csl], t1, t2)
                    else:
                        h = m - 2 * NH
                        vtr = axp.tile([P, TCH], f16, name="vtr", tag="vtr",
                                       bufs=2)
                        nc.scalar.copy(vtr, qk)
                        for jj in range(TCH // P):
                            kt = cc * (TCH // P) + jj
                            nc.sync.dma_start(
                                v_sb[half][h][:, kt * P:(kt + 1) * P],
                                vtr[:, jj * P:(jj + 1) * P], transpose=True)
        psA.release()
        axp.release()
        awp.release()

        # B/C pools; mwp_w (MLP weights) under attp so attp can pop first
        psC = tc.alloc_tile_pool(name="psC", bufs=1, space="PSUM")
        psB = tc.alloc_tile_pool(name="psB", bufs=1, space="PSUM")
        mwp_w = tc.alloc_tile_pool(name="mwp_w", bufs=1, side="right")
        attp = tc.alloc_tile_pool(name="attp", bufs=1, side="right")
        wo_sb = cpool.tile([P, NH, D], f16, name="wo_sb", tag="wo_sb")
        nc.sync.dma_start(wo_sb, wo[:, :, :])

        o_q = {}

        def stage_B_attn(b_, qc):
            """Attention (S^T) for one 512 chunk — SBUF-only, no DMA."""
            gc = b_ * CC + qc
            nkt = 4 * (qc + 1)
            # scores^T + exp, both heads
            eT = [[], []]
            for h in range(NH):
                for kt in range(nkt):
                    sc = psB.tile([P, TCH], f32, name="sc", tag="sc", bufs=2)
                    nc.tensor.matmul(
                        sc, qk_f[b_][NH + h][:, kt * P:(kt + 1) * P],
                        qk_f[b_][h][:, qc * TCH:(qc + 1) * TCH],
                        start=True, stop=True)
                    j = kt - qc * 4
                    e = attp.tile([P, TCH], f16, name="e", tag="e", bufs=16)
                    if j >= 0:
                        nc.vector.tensor_add(sc[:, j * P:(j + 1) * P],
                                             sc[:, j * P:(j + 1) * P],
                                             mask_sb)
                        if j > 0:
                            nc.vector.memset(e[:, 0:j * P], 0.0)
                        nc.scalar.activation(e[:, j * P:], sc[:, j * P:],
                                             AF.Exp, bias=ebias[:, :],
                                             scale=1.0)
                    else:
                        nc.scalar.activation(e, sc, AF.Exp, bias=ebias[:, :],
                                             scale=1.0)
                    eT[h].append(e)
            # rowsum + PV + normalize per head
            o_chunk = []
            for h in range(NH):
                rs = psB.tile([1, TCH], f32, name="rs", tag="rs", bufs=1)
                for kt in range(nkt):
                    nc.tensor.matmul(rs, ones_k, eT[h][kt],
                                     start=(kt == 0), stop=(kt == nkt - 1))
                op = psB.tile([P, TCH], f32, name="op", tag="op", bufs=1)
                for kt in range(nkt):
                    nc.tensor.matmul(op, v_sb[b_][h][:, kt * P:(kt + 1) * P],
                                     eT[h][kt],
                                     start=(kt == 0), stop=(kt == nkt - 1))
                rcp = cpool.tile([1, TCH], f32, name="rcp", tag="rcp", bufs=2)
                nc.vector.reciprocal(rcp, rs)
                rcp16 = cpool.tile([1, TCH], f16, name="rcp16", tag="rcp16",
                                   bufs=2)
                nc.vector.tensor_scalar_mul(rcp16, rcp, 1.0)
                rbp = psB.tile([P, TCH], f32, name="rbp", tag="sc", bufs=2)
                nc.tensor.matmul(rbp, ones_1, rcp16, start=True, stop=True)
                rbv = attp.tile([P, TCH], f16, name="rbv", tag="rbv", bufs=2)
                nc.vector.tensor_scalar_mul(rbv, rbp, 1.0)
                ob = attp.tile([P, TCH], f16, name="ob", tag="ob", bufs=6)
                nc.vector.tensor_tensor(ob, op, rbv, ALU.mult)
                o_chunk.append(ob)
            o_q[gc] = o_chunk

        def stage_B_wo(b_, qc):
            """Wo partials + x/8 fold, quarter-staged writes to p1."""
            gc = b_ * CC + qc
            o_chunk = o_q.pop(gc)
            for mq in range(4):
                xo8c = attp.tile([P, 4, TCH], f16, name="xo8c", tag="xo8c",
                                 bufs=2)
                nc.sync.dma_start(
                    xo8c, xo8.ap()[mq * 4:(mq + 1) * 4, :,
                                   gc * TCH:(gc + 1) * TCH]
                    .rearrange("k p t -> p k t"))
                p1s = attp.tile([P, 4, TCH], f16, name="p1s", tag="p1s",
                                bufs=3)
                for mi in range(4):
                    mout = mq * 4 + mi
                    wop = psB.tile([P, TCH], f32, name="wop", tag="wop",
                                   bufs=2)
                    for h in range(NH):
                        nc.tensor.matmul(
                            wop, wo_sb[:, h, mout * P:(mout + 1) * P],
                            o_chunk[h], start=(h == 0), stop=(h == NH - 1))
                    nc.vector.tensor_tensor(p1s[:, mi, :], wop,
                                            xo8c[:, mi, :], ALU.add)
                nc.sync.dma_start(
                    p1[gc // GPA].ap()[mq * 4:(mq + 1) * 4, gc % GPA, :, :]
                    .rearrange("k p t -> p k t"), p1s)

        def stage_AR1(ar):
            nc.gpsimd.collective_compute(
                "AllReduce", ALU.add, replica_groups=rgroups,
                ins=[p1[ar].ap()], outs=[a1[ar].ap()])

        def stage_C(gc):
            """x1 = a1 chunk -> rms2 -> write normalized xn2 tiles."""
            a1c = cpool.tile([P, KD, TCH], f16, name="a1c", tag="a1c", bufs=1)
            nc.sync.dma_start(
                a1c, a1[gc // GPA].ap()[:, gc % GPA, :, :]
                .rearrange("k p t -> p k t"))
            ssq = psC.tile([1, TCH], f32, name="ssq", tag="ssq", bufs=1)
            for i in range(KD):
                x2t = cpool.tile([P, TCH], f16, name="x2t", tag="x2t", bufs=2)
                nc.vector.tensor_mul(x2t, a1c[:, i, :], a1c[:, i, :])
                nc.tensor.matmul(ssq, ones_k, x2t,
                                 start=(i == 0), stop=(i == KD - 1))
            srt = cpool.tile([1, TCH], f32, name="srt", tag="srt", bufs=1)
            nc.scalar.activation(srt, ssq, AF.Sqrt, bias=eps1[:, :],
                                 scale=1.0 / D)
            rr2 = cpool.tile([1, TCH], f32, name="rr2", tag="rr2", bufs=1)
            nc.vector.reciprocal(rr2, srt)
            rr216 = cpool.tile([1, TCH], f16, name="rr216", tag="rr216",
                               bufs=2)
            nc.vector.tensor_scalar_mul(rr216, rr2, 1.0)
            rbp2 = psC.tile([P, TCH], f32, name="rbp2", tag="rbp2", bufs=1)
            nc.tensor.matmul(rbp2, ones_1, rr216, start=True, stop=True)
            rsb = cpool.tile([P, TCH], f16, name="rsb", tag="rsb", bufs=2)
            nc.vector.tensor_scalar_mul(rsb, rbp2, 1.0)
            for kq in range(4):
                xnc = cpool.tile([P, 4, TCH], f16, name="xnc", tag="xnc",
                                 bufs=2)
                for ki in range(4):
                    i = kq * 4 + ki
                    nc.gpsimd.tensor_tensor(xnc[:, ki, :], a1c[:, i, :],
                                            rsb, ALU.mult)
                nc.sync.dma_start(
                    xn2d.ap()[gc, kq * 4:(kq + 1) * 4, :, :]
                    .rearrange("k p t -> p k t"), xnc)

        xn2q = {}

        def load_D(cc):
            xc = dwp2.tile([P, KD, TCH], f16, name="xn2c", tag="xn2c",
                           bufs=2)
            nc.sync.dma_start(
                xc, xn2d.ap()[cc, :, :, :].rearrange("k p t -> p k t"))
            xn2q[cc] = xc

        def stage_D(cc):
            """MLP for one 512-token chunk; p2 partial (mlp only)."""
            xc = xn2q.pop(cc)
            acs = []
            for fm in range(KF):
                gp = psD.tile([P, TCH], f32, name="gp", tag="gp", bufs=2)
                for i in range(KD):
                    nc.tensor.matmul(gp, wg_sb[:, i, fm * P:(fm + 1) * P],
                                     xc[:, i, :],
                                     start=(i == 0), stop=(i == KD - 1))
                up = psD.tile([P, TCH], f32, name="up", tag="up", bufs=2)
                for i in range(KD):
                    nc.tensor.matmul(up, wu_sb[:, i, fm * P:(fm + 1) * P],
                                     xc[:, i, :],
                                     start=(i == 0), stop=(i == KD - 1))
                gss = dwp1.tile([P, TCH], f16, name="gss", tag="gss", bufs=2)
                nc.scalar.activation(gss, gp, AF.Silu)
                ac = dwp2.tile([P, TCH], f16, name="ac", tag="ac",
                               bufs=KF + 1)
                nc.vector.tensor_tensor(ac, gss, up, ALU.mult)
                acs.append(ac)
            for mq in range(4):
                p2s = dwp1.tile([P, 4, TCH], f16, name="p2s", tag="p2s",
                               bufs=2)
                for mi in range(4):
                    mout = mq * 4 + mi
                    dp = psD.tile([P, TCH], f32, name="dp", tag="dp", bufs=2)
                    for fi in range(KF):
                        nc.tensor.matmul(
                            dp, wd_sb[:, fi, mout * P:(mout + 1) * P],
                            acs[fi], start=(fi == 0), stop=(fi == KF - 1))
                    nc.vector.tensor_scalar_mul(p2s[:, mi, :], dp, 1.0)
                nc.sync.dma_start(
                    p2[cc].ap()[mq * 4:(mq + 1) * 4, :, :]
                    .rearrange("k p t -> p k t"), p2s)

        def stage_AR2(cc):
            nc.gpsimd.collective_compute(
                "AllReduce", ALU.add, replica_groups=rgroups,
                ins=[p2[cc].ap()], outs=[a2[cc].ap()])

        def stage_Y(cc):
            """y = a1 + a2 for one 512 chunk -> yT."""
            for hh in range(8):
                ksl = slice(hh * 2, (hh + 1) * 2)
                ay1 = ypool.tile([P, 2, TCH], f16, name="ay1", tag="ay1",
                                 bufs=1)
                nc.sync.dma_start(
                    ay1, a1[cc // GPA].ap()[ksl, cc % GPA, :, :]
                    .rearrange("k p t -> p k t"))
                ay2 = ypool.tile([P, 2, TCH], f16, name="ay2", tag="ay2",
                                 bufs=1)
                nc.sync.dma_start(
                    ay2, a2[cc].ap()[ksl, :, :].rearrange("k p t -> p k t"))
                yc = ypool.tile([P, 2, TCH], f16, name="yc", tag="yc",
                                bufs=1)
                nc.gpsimd.tensor_tensor(yc, ay1, ay2, ALU.add)
                nc.sync.dma_start(
                    yT.ap()[cc, ksl, :, :].rearrange("k p t -> p k t"), yc)

        def stage_Y_big(cc):
            """Final-chunk y using the (now idle) wide xn2c buffers."""
            ay1 = dwp2.tile([P, KD, TCH], f16, name="yb1", tag="xn2c",
                            bufs=2)
            nc.sync.dma_start(
                ay1, a1[cc // GPA].ap()[:, cc % GPA, :, :]
                .rearrange("k p t -> p k t"))
            ay2 = dwp2.tile([P, KD, TCH], f16, name="yb2", tag="xn2c",
                            bufs=2)
            nc.sync.dma_start(
                ay2, a2[cc].ap()[:, :, :].rearrange("k p t -> p k t"))
            nc.gpsimd.tensor_tensor(ay1, ay1, ay2, ALU.add)
            nc.sync.dma_start(
                yT.ap()[cc, :, :, :].rearrange("k p t -> p k t"), ay1)

        # ================= pipelined issue order =================
        # AR1 rings (4MB, ~70us of saturated DMA engines) are covered by
        # the next two chunks' DMA-free attention compute; Wo evictions
        # (DMA-dependent) are deferred until after each ring.
        stage_B_attn(0, 0)
        stage_B_wo(0, 0)
        stage_B_attn(0, 1)
        stage_B_wo(0, 1)
        stage_AR1(0)
        stage_B_attn(0, 2)
        # kick gate/up weight loads
        wg_sb = mwp_w.tile([P, KD, FH], f16, name="wg_sb", tag="wg_sb")
        nc.sync.dma_start(wg_sb, wg[:, :, :])
        wu_sb = mwp_w.tile([P, KD, FH], f16, name="wu_sb", tag="wu_sb")
        nc.sync.dma_start(wu_sb, wu[:, :, :])
        stage_B_attn(0, 3)
        stage_B_wo(0, 2)
        stage_B_wo(0, 3)
        stage_AR1(1)
        persist[0].release()

        stage_B_attn(1, 0)
        stage_B_attn(1, 1)
        stage_B_wo(1, 0)
        stage_B_wo(1, 1)
        stage_AR1(2)
        stage_C(0)
        stage_C(1)
        stage_B_attn(1, 2)
        stage_B_attn(1, 3)
        stage_B_wo(1, 2)
        stage_B_wo(1, 3)
        stage_AR1(3)
        stage_C(2)
        stage_C(3)
        persist[1].release()
        psB.release()
        attp.release()
        psD = tc.alloc_tile_pool(name="psD", bufs=1, space="PSUM")
        dwp1 = tc.alloc_tile_pool(name="dwp1", bufs=1, side="right")
        dwp2 = tc.alloc_tile_pool(name="dwp2", bufs=1, side="right")
        ypool = tc.alloc_tile_pool(name="ypool", bufs=1, side="right")
        wd_sb = dwp1.tile([P, KF, D], f16, name="wd_sb", tag="wd_sb")
        nc.sync.dma_start(wd_sb, wd[:, :, :])

        load_D(0)
        load_D(1)
        stage_D(0)
        stage_AR2(0)
        stage_C(4)
        load_D(2)
        stage_D(1)
        stage_AR2(1)
        stage_C(5)
        load_D(3)
        stage_D(2)
        stage_AR2(2)
        stage_C(6)
        load_D(4)
        stage_D(3)
        stage_AR2(3)
        stage_C(7)
        load_D(5)
        stage_Y(0)
        stage_D(4)
        stage_AR2(4)
        load_D(6)
        stage_Y(1)
        stage_D(5)
        stage_AR2(5)
        load_D(7)
        stage_Y(2)
        stage_D(6)
        stage_AR2(6)
        stage_Y(3)
        stage_Y(4)
        stage_D(7)
        stage_AR2(7)
        stage_Y(5)
        stage_Y(6)
        stage_Y_big(7)

        psD.release()
        psC.release()
        ypool.release()
        dwp2.release()
        dwp1.release()
        mwp_w.release()
        cpool.release()

    nc.compile()
    return nc


# ---------------- host side ----------------

_BUILT = {}


def _get_program():
    if "full" not in _BUILT:
        _BUILT["full"] = build_decoder()
    return _BUILT["full"]


def _tile_k(a2d):
    """[D, N] -> [KD, P, N] contiguous f16."""
    return np.ascontiguousarray(
        a2d.reshape(KD, P, a2d.shape[1])).astype(np.float16)


def _host_prep(x, position_ids, Wq, Wk, Wv, Wo, Wg, Wu, Wd, g1, g2):
    xf = np.asarray(x, np.float32).reshape(N, D)
    rs1 = 1.0 / np.sqrt(np.mean(xf * xf, axis=1, keepdims=True) + EPS)
    xn = (xf * rs1).T                       # [D, N] normalized (g1 in weights)
    xo8f = (xf.T / 8.0)                     # [D, N]
    xn16 = _tile_k(xn)
    xo8t = _tile_k(xo8f)

    pos = np.asarray(position_ids).reshape(-1).astype(np.float32)
    inv_freq = (1.0 / (BASE ** (np.arange(0, HD, 2, dtype=np.float32) / HD)))
    ang = pos[:, None] * inv_freq[None, :]
    cos_f = np.concatenate([np.cos(ang), np.cos(ang)], axis=1)   # [N, HD]
    sin_f = np.concatenate([np.sin(ang), np.sin(ang)], axis=1)
    sign = np.concatenate([-np.ones(HD // 2, np.float32),
                           np.ones(HD // 2, np.float32)])
    s = 1.0 / math.sqrt(HD)
    cqt = np.ascontiguousarray(cos_f.T * s).astype(np.float16)
    sqt = np.ascontiguousarray((sin_f * sign).T * s).astype(np.float16)
    ckt = np.ascontiguousarray(cos_f.T).astype(np.float16)
    skt = np.ascontiguousarray((sin_f * sign).T).astype(np.float16)

    ii, jj = np.indices((P, P))
    maskv = np.where(ii > jj, np.float32(-10000.0), np.float32(0.0))

    g1f = np.asarray(g1, np.float32)[:, None]
    g2f = np.asarray(g2, np.float32)[:, None]
    wqs = (g1f * np.asarray(Wq, np.float32)).astype(np.float16)
    wks = (g1f * np.asarray(Wk, np.float32)).astype(np.float16)
    wvs = (g1f * np.asarray(Wv, np.float32)).astype(np.float16)
    wgs = (g2f * np.asarray(Wg, np.float32)).astype(np.float16)
    wus = (g2f * np.asarray(Wu, np.float32)).astype(np.float16)
    wds = np.asarray(Wd, np.float32).astype(np.float16)
    wos = np.asarray(Wo, np.float32).astype(np.float16)

    def tile_w(w):   # [D_in, M] -> [P, D_in//P, M]
        din, m = w.shape
        return np.ascontiguousarray(
            w.reshape(din // P, P, m).transpose(1, 0, 2))

    in_maps = []
    for i in range(NCORES):
        qs, fs = slice(i * DH, (i + 1) * DH), slice(i * FH, (i + 1) * FH)
        wq_c = np.concatenate([wqs[:, qs], wks[:, qs], wvs[:, qs]], axis=1)
        wo_c = wos[qs, :]                    # [DH, D]
        in_maps.append({
            "xn16": xn16, "xo8": xo8t,
            "cq": cqt, "sq": sqt, "ck": ckt, "sk": skt,
            "maskd": maskv,
            "wqkv": tile_w(wq_c),
            "wo": np.ascontiguousarray(
                wo_c.reshape(NH, P, D).transpose(1, 0, 2)),
            "wg": tile_w(wgs[:, fs]),
            "wu": tile_w(wus[:, fs]),
            "wd": tile_w(wds[fs, :]),
        })
    return in_maps


FULL_CFG = dict(B=B, T=T, D=D, H=H, FF=FF)


def run(cfg, inputs, **run_kwargs):
    nc = _get_program()
    in_maps = _host_prep(**inputs)
    res = bass_utils.run_bass_kernel_spmd(
        nc, in_maps, core_ids=list(range(NCORES)), **run_kwargs)
    yt = res.results[0]["yT"]               # [NTC, KD, P, TCH] f16
    y = np.transpose(yt.astype(np.float32), (0, 3, 1, 2)).reshape(N, D)
    return y.reshape(B, T, D), res


def kernel(**inputs):
    y, _ = run(FULL_CFG, inputs)
    return y
